# revision 22
# baseline (speedup 1.0000x reference)
"""Trainium2 Bass kernel for nn_MHSA_37821482008969 (2D rel-pos MHSA).

Strategy: data-parallel over batch (16 batches -> 8 cores x 2). Per (batch,
head) unit, attention is computed fully transposed: S^T = K^T@Q tiles with
y (keys) on partitions, so softmax-normalization sums come from a ones-vector
matmul on PE, the attn matmul needs no transposes of exp(S), and the output
lands directly in the channel-major layout the conv output wants.

Rel-pos biases are folded into the logits accumulation as one extra K=64
matmul per tile: lhsT is a constant 0/1 selector, rhs is the skewed rel-logit
table built via a DRAM round-trip (regular strided APs implement the
rel->abs skew) plus two PE transposes for the width term.

Perf structure (441us -> 187us over 13 iterations):
- both batches' projections emitted up front (rel-table building lagged one
  head behind Q/K so PE never waits on PSUM->SBUF casts); trailing proj
  pieces of batch 1 woven between the first attention blocks.
- 16 attention (head, x-block) blocks run as a software pipeline with the
  normalization tail lagged two blocks:
    iter j: logits+exp(j) | attn+sums+recip(j-1) | bcast+mul+store(j-2)
- logits st tiles are [128,512] x 5 PSUM banks so slot recycling decouples
  from ACT exp latency; bias sel-matmuls run as 64-row row-tiled pairs
  (tile_position (0,0)/(64,0)) -- two K=64 matmuls stream concurrently;
  rel-height matmuls use column tiling (M=63) the same way.
- softmax sums via DVE/GpSimd bf16 add-tree (1 PE matmul instead of 8 per
  block; last two blocks accumulate on PE directly to shorten the tail);
  reciprocal via the fast custom-DVE approximation; the 1/sum row is
  broadcast across partitions by a stride-0 DRAM-read DMA (no PE matmul,
  PE-matmul fallback for the final two blocks).
- the rel->abs W-skew transposes run on [128,32] slices whose [32,128]
  outputs land partition-aligned with the bias rows (plain ACT copies
  replace 64 partition-crossing assembly DMAs); rel-width logits for all 8
  x-tiles go into one PSUM bank, one cast, one 3D-strided DMA.
- DMA dispatch spread across sync/gpsimd queues (the ACT queue carries no
  DMAs so exps are never head-of-line blocked); wq/x chunks interleaved at
  the queue head so the first matmul starts ~10us in; exp table-set warmed
  during projections.
- exp per [128,512] PSUM bank tile; output stored bf16, host upcasts.
All matmul operands are bf16 (fp32 PSUM accumulation); softmax skips the
row-max subtraction (logits are ~N(0,1), |logit| < 7, exp is safe in fp32).
"""
import numpy as np
import ml_dtypes

import concourse.bass as bass
import concourse.mybir as mybir
import concourse.tile as tile
import concourse.bacc as bacc
from concourse.bass_utils import run_bass_kernel_spmd

bf16 = ml_dtypes.bfloat16
FP32 = mybir.dt.float32
BF16 = mybir.dt.bfloat16

HEADS, D, F, DIM = 4, 128, 32, 512
L = F * F           # 1024
B_PER_CORE = 2
N_CORES = 8
AF = mybir.ActivationFunctionType

_cache = {}


def _build():
    nc = bacc.Bacc("TRN2", target_bir_lowering=False, debug=False,
                   num_devices=N_CORES)
    xin = nc.dram_tensor("xin", [B_PER_CORE, 4, 128, L], BF16, kind="ExternalInput").ap()
    wqt = nc.dram_tensor("wqt", [4, 128, DIM], BF16, kind="ExternalInput").ap()
    wkt = nc.dram_tensor("wkt", [4, 128, DIM], BF16, kind="ExternalInput").ap()
    wvt = nc.dram_tensor("wvt", [4, 128, DIM], BF16, kind="ExternalInput").ap()
    relwt = nc.dram_tensor("relwt", [128, 63], BF16, kind="ExternalInput").ap()
    relht = nc.dram_tensor("relht", [128, 63], BF16, kind="ExternalInput").ap()
    sel = nc.dram_tensor("sel", [128, 4 * 128], BF16, kind="ExternalInput").ap()
    ones_col = nc.dram_tensor("ones_col", [128, 1], BF16, kind="ExternalInput").ap()
    ones_row = nc.dram_tensor("ones_row", [1, 128], BF16, kind="ExternalInput").ap()
    ident = nc.dram_tensor("ident", [128, 128], BF16, kind="ExternalInput").ap()
    out = nc.dram_tensor("out", [B_PER_CORE, DIM, L], BF16, kind="ExternalOutput").ap()

    from contextlib import ExitStack
    ctx = ExitStack()
    with tile.TileContext(nc) as tc, ctx:
        consts = ctx.enter_context(tc.tile_pool(name="consts", bufs=1))
        xpool = ctx.enter_context(tc.tile_pool(name="xpool", bufs=2))
        qkpool = ctx.enter_context(tc.tile_pool(name="qkpool", bufs=2))
        vtpool = ctx.enter_context(tc.tile_pool(name="vtpool", bufs=2))
        relpool = ctx.enter_context(tc.tile_pool(name="relpool", bufs=4))
        biaspool = ctx.enter_context(tc.tile_pool(name="biaspool", bufs=2))
        ptpool = ctx.enter_context(tc.tile_pool(name="ptpool", bufs=2))
        accpool = ctx.enter_context(tc.tile_pool(name="accpool", bufs=2))
        outpool = ctx.enter_context(tc.tile_pool(name="outpool", bufs=3))
        psA = ctx.enter_context(tc.tile_pool(name="psA", bufs=5, space="PSUM"))
        psB = ctx.enter_context(tc.tile_pool(name="psB", bufs=2, space="PSUM"))
        psS = ctx.enter_context(tc.tile_pool(name="psS", bufs=1, space="PSUM"))
        dramw = ctx.enter_context(tc.tile_pool(name="dramw", bufs=4, space="DRAM"))
        dramh = ctx.enter_context(tc.tile_pool(name="dramh", bufs=4, space="DRAM"))
        dramr = ctx.enter_context(tc.tile_pool(name="dramr", bufs=2, space="DRAM"))

        # ---- constants: spread DMA dispatch across engines so the first
        #      projection matmuls aren't stuck behind one serial queue ----
        def cload(eng, ap, shape, tag):
            t = consts.tile(shape, ap.dtype, tag=tag, name=tag)
            eng.dma_start(t[:], ap)
            return t
        x_all = [[None] * 4 for _ in range(B_PER_CORE)]
        xb0 = xpool.tile([128, 4 * L], BF16, tag="xb", name="xb0")
        wq_sb = []
        for c in range(4):
            wq_sb.append(cload(nc.sync, wqt[c], [128, DIM], f"wq{c}"))
            nc.sync.dma_start(xb0[:, c * L:(c + 1) * L], xin[0, c])
            x_all[0][c] = xb0[:, c * L:(c + 1) * L]
        xb1 = xpool.tile([128, 4 * L], BF16, tag="xb", name="xb1")
        xf = xb1[:]
        xs = xin[1].flatten()
        nc.sync.dma_start(
            bass.AP(xf.tensor, xf.offset, [[4 * L, 128], [L, 4], [1, L]]),
            bass.AP(xs.tensor, xs.offset, [[L, 128], [128 * L, 4], [1, L]]))
        for c in range(4):
            x_all[1][c] = xb1[:, c * L:(c + 1) * L]
        wk_sb = [cload(nc.gpsimd, wkt[c], [128, DIM], f"wk{c}") for c in range(4)]
        relw_sb = cload(nc.gpsimd, relwt, [128, 63], "relw")
        relh_sb = cload(nc.gpsimd, relht, [128, 63], "relh")
        wv_sb = [cload(nc.gpsimd, wvt[c], [128, DIM], f"wv{c}") for c in range(4)]
        sel_sb = cload(nc.gpsimd, sel, [128, 4 * 128], "sel")
        ones_c = cload(nc.gpsimd, ones_col, [128, 1], "onesc")
        ones_r = cload(nc.gpsimd, ones_row, [1, 128], "onesr")
        id_sb = cload(nc.gpsimd, ident, [128, 128], "ident")
        # warm the exp table-set on ACT during the projection phase so the
        # first attention block does not pay the ~2.7us ACT_TABLE_LOAD
        warm = consts.tile([1, 1], FP32, tag="warm", name="warm")
        nc.scalar.activation(warm[:], ones_c[0:1, 0:1], AF.Exp)

        q_all = [[None] * HEADS for _ in range(B_PER_CORE)]
        k_all = [[None] * HEADS for _ in range(B_PER_CORE)]
        vt_all = [[None] * 4 for _ in range(B_PER_CORE)]
        bias_all = [[None] * HEADS for _ in range(B_PER_CORE)]
        wst2_all = [[None] * HEADS for _ in range(B_PER_CORE)]

        def emit_qk(b, h):
            x_sb = x_all[b]
            for dst_list, w in ((q_all, wq_sb), (k_all, wk_sb)):
                dst = qkpool.tile([128, L], BF16,
                                  tag=("q" if dst_list is q_all else "k") + str(h),
                                  name=f"qk{b}_{h}")
                for nn in range(2):
                    ps = psA.tile([128, DIM], FP32, tag="st", name=f"qk{b}_{h}")
                    sl = slice(nn * 512, (nn + 1) * 512)
                    for c in range(4):
                        lhsT = w[c][:, h * 128:(h + 1) * 128]
                        nc.tensor.matmul(ps[:], lhsT, x_sb[c][:, sl],
                                         start=(c == 0), stop=(c == 3))
                    nc.vector.tensor_copy(dst[:, sl], ps[:])
                dst_list[b][h] = dst

        def emit_rel(b, h):
            q_sb = q_all[b][h]
            # ---- rel width logits RW[x, m]: 8 x-tiles into one PSUM bank ----
            psrw = psB.tile([128, 512], FP32, tag="attn", name=f"rw{b}_{h}")
            for j in range(8):
                nc.tensor.matmul(psrw[:, j * 64:j * 64 + 63],
                                 q_sb[:, j * 128:(j + 1) * 128],
                                 relw_sb[:], start=True, stop=True)
            rwall = relpool.tile([128, 512], BF16, tag="rw", name=f"rw{b}_{h}")
            nc.vector.tensor_copy(rwall[:], psrw[:])
            skw = dramw.tile([L, 64], BF16, tag="skw", name=f"skw{b}_{h}")
            rwf = rwall[:].flatten()
            swf = skw[:].flatten()
            nc.gpsimd.dma_start(
                bass.AP(swf.tensor, swf.offset, [[64, 128], [8192, 8], [1, 63]]),
                bass.AP(rwf.tensor, rwf.offset, [[512, 128], [64, 8], [1, 63]]))
            # ---- rel height logits RH_T[m, x] -> DRAM ([64, L]) ----
            rh = relpool.tile([64, L], BF16, tag="rh", name=f"rh{b}_{h}")
            ps2 = psA.tile([128, DIM], FP32, tag="st", name=f"rh{b}_{h}")
            nc.tensor.matmul(ps2[0:63, :], relh_sb[:], q_sb[:, 0:512],
                             start=True, stop=True)
            nc.tensor.matmul(ps2[64:127, :], relh_sb[:], q_sb[:, 512:1024],
                             start=True, stop=True)
            nc.vector.tensor_copy(rh[0:63, 0:512], ps2[0:63, :])
            nc.vector.tensor_copy(rh[0:63, 512:1024], ps2[64:127, :])
            skh = dramh.tile([64, L], BF16, tag="skh", name=f"skh{b}_{h}")
            nc.gpsimd.dma_start(skh[0:63, :], rh[0:63, :])

            # ---- skewed reads: W -> wst2 (pre-transpose), H -> bias rows 32:64
            bias_rhs = biaspool.tile([128, L], BF16, tag=f"bias{h}",
                                     name=f"bias{b}_{h}")
            bias_all[b][h] = bias_rhs
            wst2 = relpool.tile([128, 256], BF16, tag="wst2", name=f"wst2{b}_{h}")
            wst2_all[b][h] = wst2
            src_flat = skw[:].flatten()
            dst_flat = wst2[:]
            for xh in range(4):
                srcap = bass.AP(src_flat.tensor, src_flat.offset + 31 + xh * 2048,
                                [[63, 32], [8192, 8], [1, 32]])
                dstap = bass.AP(dst_flat.tensor, dst_flat.offset + xh * 32 * 256,
                                [[256, 32], [32, 8], [1, 32]])
                nc.sync.dma_start(dstap, srcap)
            hsrc_flat = skh[:].flatten()
            hsrc = bass.AP(hsrc_flat.tensor, hsrc_flat.offset,
                           [[1024, 32], [1056, 32], [1, 32]])
            hdst_flat = bias_rhs[:]
            hdst = bass.AP(hdst_flat.tensor, hdst_flat.offset + 32 * 1024,
                           [[1024, 32], [32, 32], [1, 32]])
            nc.sync.dma_start(hdst, hsrc)

        def emit_vt(b, p2s):
            # V^T pairs: vt2[p][y(128), (yt half)*512 + d] for all 4 heads
            for p2 in p2s:
                vt2 = vtpool.tile([128, 2 * DIM], BF16, tag=f"vt{p2}",
                                  name=f"v{b}_{p2}")
                for half in range(2):
                    yt = p2 * 2 + half
                    sl = slice(half * 512, (half + 1) * 512)
                    ps = psA.tile([128, DIM], FP32, tag="st", name=f"v{b}_{p2}")
                    for c in range(4):
                        nc.tensor.matmul(ps[:],
                                         x_all[b][c][:, yt * 128:(yt + 1) * 128],
                                         wv_sb[c][:], start=(c == 0), stop=(c == 3))
                    nc.vector.tensor_copy(vt2[:, sl], ps[:])
                vt_all[b][p2] = vt2

        def emit_wtrans(b, hs):
            # transpose wst2 in [128,32] slices: output [32,128] lands on
            # partitions 0:32 = exactly the bias_w rows -> plain ACT copies,
            # no partition-crossing assembly DMAs needed
            for h in hs:
                wst2 = wst2_all[b][h]
                bias_rhs = bias_all[b][h]
                for s in range(8):
                    tps = psB.tile([32, 128], BF16, tag="attn", name=f"tp{b}_{h}")
                    nc.tensor.transpose(tps[:], wst2[:, s * 32:(s + 1) * 32],
                                        id_sb[:])
                    nc.scalar.activation(bias_rhs[0:32, s * 128:(s + 1) * 128],
                                         tps[:], AF.Identity)
                nc.sync.dma_start(bias_rhs[64:128, :], bias_rhs[0:64, :])

        # ---- attention block list (emitted interleaved with proj below) ----
        blocks = [(b, h, n)
                  for b in range(B_PER_CORE)
                  for h in range(HEADS)
                  for n in range(2)]
        S = [None] * len(blocks)

        def emit_logits(j):
            b, h, n = blocks[j]
            q_sb, k_sb = q_all[b][h], k_all[b][h]
            bias_rhs = bias_all[b][h]
            nsl = slice(n * 512, (n + 1) * 512)
            pts = []
            for g in range(2):
                sts = []
                for i in range(4):
                    yt = g * 4 + i
                    st = psA.tile([128, 512], FP32, tag="st", name=f"st{j}_{yt}")
                    nc.tensor.matmul(st[:], k_sb[:, yt * 128:(yt + 1) * 128],
                                     q_sb[:, nsl], start=True, stop=False)
                    sts.append(st)
                for i in range(2):
                    p = g * 2 + i
                    psl = slice(p * 128, (p + 1) * 128)
                    nc.tensor.matmul(sts[2 * i][:], sel_sb[0:64, psl],
                                     bias_rhs[0:64, nsl], start=False, stop=True)
                    nc.tensor.matmul(sts[2 * i + 1][:], sel_sb[64:128, psl],
                                     bias_rhs[64:128, nsl], start=False, stop=True)
                for i in range(4):
                    yt = g * 4 + i
                    pt = ptpool.tile([128, 512], BF16, tag=f"pt{yt}",
                                     name=f"pt{j}_{yt}")
                    nc.scalar.activation(pt[:], sts[i][:], AF.Exp)
                    pts.append(pt)
            if j < len(blocks) - 2:
                # bf16 add-tree: acc[y, x] = sum over the 8 yt tiles
                tadd = []
                for a in range(4):
                    t = accpool.tile([128, 512], BF16, tag=f"t{a}", name=f"t{a}_{j}")
                    eng = nc.gpsimd if a % 2 == 0 else nc.vector
                    eng.tensor_add(t[:], pts[2 * a][:], pts[2 * a + 1][:])
                    tadd.append(t)
                u0 = accpool.tile([128, 512], BF16, tag="u0", name=f"u0_{j}")
                nc.vector.tensor_add(u0[:], tadd[0][:], tadd[1][:])
                u1 = accpool.tile([128, 512], BF16, tag="u1", name=f"u1_{j}")
                nc.vector.tensor_add(u1[:], tadd[2][:], tadd[3][:])
                acc = accpool.tile([128, 512], BF16, tag="acc", name=f"acc_{j}")
                nc.vector.tensor_add(acc[:], u0[:], u1[:])
                S[j] = {"pts": pts, "acc": acc}
            else:
                S[j] = {"pts": pts}

        def emit_attn(j):
            b, h, n = blocks[j]
            s = S[j]
            attn = psB.tile([128, 512], FP32, tag="attn", name=f"at{j}")
            for yt in range(8):
                nc.tensor.matmul(attn[:],
                                 vt_all[b][yt // 2][:, (yt % 2) * 512 + h * 128:
                                                    (yt % 2) * 512 + (h + 1) * 128],
                                 s["pts"][yt][:],
                                 start=(yt == 0), stop=(yt == 7))
            sums = psS.tile([1, 512], FP32, tag="sums", name=f"sm{j}")
            if "acc" in s:
                nc.tensor.matmul(sums[:], ones_c[:], s["acc"][:],
                                 start=True, stop=True)
            else:
                for yt in range(8):
                    nc.tensor.matmul(sums[:], ones_c[:], s["pts"][yt][:],
                                     start=(yt == 0), stop=(yt == 7))
            recipf = accpool.tile([1, 512], FP32, tag="recipf", name=f"rf{j}")
            nc.vector.reciprocal_approx_fast(recipf[:], sums[:])
            recipb = accpool.tile([1, 512], BF16, tag="recipb", name=f"rb{j}")
            nc.vector.tensor_copy(recipb[:], recipf[:])
            s["attn"] = attn
            if j >= len(blocks) - 2:
                s["recipb"] = recipb
            else:
                drc = dramr.tile([1, 512], BF16, tag="drc", name=f"drc{j}")
                nc.gpsimd.dma_start(drc[:], recipb[:])
                s["drc"] = drc

        def emit_norm(j):
            b, h, n = blocks[j]
            s = S[j]
            bcs = outpool.tile([128, 512], BF16, tag="bcs", name=f"bcs{j}")
            if "drc" in s:
                df = s["drc"][:].flatten()
                nc.gpsimd.dma_start(bcs[:],
                                    bass.AP(df.tensor, df.offset,
                                            [[0, 128], [1, 512]]))
            else:
                bc = psS.tile([128, 512], FP32, tag="sums", name=f"bc{j}")
                nc.tensor.matmul(bc[:], ones_r[:], s["recipb"][:],
                                 start=True, stop=True)
                nc.scalar.activation(bcs[:], bc[:], AF.Identity)
            o = outpool.tile([128, 512], BF16, tag="o", name=f"o{j}")
            with nc.allow_low_precision(reason="bf16 output"):
                nc.vector.tensor_mul(o[:], s["attn"][:], bcs[:])
            nc.sync.dma_start(out[b, h * 128:(h + 1) * 128, n * 512:(n + 1) * 512],
                              o[:])
            S[j] = None

        # ---- emission schedule: proj(b0), proj(b1) trailing pieces woven
        #      between the first attention blocks of b0 ----
        emit_qk(0, 0)
        for h in range(1, HEADS):
            emit_qk(0, h)
            emit_rel(0, h - 1)
        emit_rel(0, HEADS - 1)
        emit_vt(0, range(4))
        emit_wtrans(0, range(HEADS))
        emit_qk(1, 0)
        for h in range(1, HEADS):
            emit_qk(1, h)
            emit_rel(1, h - 1)
        emit_rel(1, HEADS - 1)

        def emit_block(j):
            emit_logits(j)
            if j >= 1:
                emit_attn(j - 1)
            if j >= 2:
                emit_norm(j - 2)

        emit_vt(1, range(0, 2))
        emit_block(0)
        emit_vt(1, range(2, 4))
        emit_block(1)
        emit_wtrans(1, range(0, 2))
        emit_block(2)
        emit_wtrans(1, range(2, 4))
        for j in range(3, len(blocks)):
            emit_block(j)
        emit_attn(len(blocks) - 1)
        emit_norm(len(blocks) - 2)
        emit_norm(len(blocks) - 1)

    nc.compile()
    return nc


def _prep_inputs(featuremap, w_qk, w_v, rel_height, rel_width):
    scale = D ** -0.5
    wqt = np.ascontiguousarray(w_qk[:DIM].T * scale).astype(bf16).reshape(4, 128, DIM)
    wkt = np.ascontiguousarray(w_qk[DIM:].T).astype(bf16).reshape(4, 128, DIM)
    wvt = np.ascontiguousarray(w_v.T).astype(bf16).reshape(4, 128, DIM)
    relwt = np.ascontiguousarray(rel_width.T).astype(bf16)
    relht = np.ascontiguousarray(rel_height.T[:, ::-1]).astype(bf16)
    yy = np.arange(128)
    sel1 = np.zeros((64, 8 * 128), np.float32)
    for yt in range(8):
        sel1[yy % 32, yt * 128 + yy] = 1.0
        sel1[32 + 31 - (yt * 4 + yy // 32), yt * 128 + yy] = 1.0
    # stacked pairs for 64-row dual matmuls: rows 0:64 = even yt, 64:128 = odd
    sel = np.zeros((128, 4 * 128), np.float32)
    for p in range(4):
        sel[0:64, p * 128:(p + 1) * 128] = sel1[:, (2 * p) * 128:(2 * p + 1) * 128]
        sel[64:128, p * 128:(p + 1) * 128] = sel1[:, (2 * p + 1) * 128:(2 * p + 2) * 128]
    sel = sel.astype(bf16)
    ones_col = np.ones((128, 1), bf16)
    ones_row = np.ones((1, 128), bf16)
    ident = np.eye(128, dtype=bf16)
    common = dict(wqt=wqt, wkt=wkt, wvt=wvt, relwt=relwt, relht=relht,
                  sel=sel, ones_col=ones_col, ones_row=ones_row, ident=ident)
    xin = featuremap.reshape(16, DIM, L).astype(bf16).reshape(
        N_CORES, B_PER_CORE, 4, 128, L)
    return [dict(common, xin=np.ascontiguousarray(xin[i])) for i in range(N_CORES)]


def kernel(featuremap, w_qk, w_v, rel_height, rel_width, _trace=False, _tmpdir=None):
    if "nc" not in _cache:
        _cache["nc"] = _build()
    nc = _cache["nc"]
    in_maps = _prep_inputs(featuremap, w_qk, w_v, rel_height, rel_width)
    res = run_bass_kernel_spmd(nc, in_maps, list(range(N_CORES)),
                               trace=_trace, tmpdir=_tmpdir)
    _cache["last_result"] = res
    full = np.concatenate([res.results[i]["out"] for i in range(N_CORES)], axis=0)
    return full.astype(np.float32).reshape(16, DIM, F, F)
